# revision 6
# baseline (speedup 1.0000x reference)
"""Causal no-head self-attention with RoPE on 8 Trainium2 NeuronCores — v2.

Sharding: core (b, h) handles batch b, queries at positions h::2 (strided).
Strided assignment makes the causal structure identical on every core
(SPMD with zero padding waste): query tile k (256 queries, spanning 512
positions) attends key slots 0..4k+3 (128 keys each).

All-bf16 datapath (tolerance 2e-2): K^T and V stay RESIDENT in SBUF
(no DRAM spill), projections/attention matmuls in bf16 (1 cyc/row,
same PE rate as f32r but half the SBUF/DMA).

Attention processed in two query halves (512 wide) so scores/PV matmuls
run at N=512; slots above the diagonal use N=256 with a zero-padded PT.
Softmax denominators via ones-column matmul; normalization folded into
the Wo-output eviction (attn @ Wo is linear in the per-query scale).
"""

import numpy as np
import sys

for _p in ("/opt/trn_rl_repo",):
    if _p not in sys.path:
        sys.path.insert(0, _p)

import concourse.bass as bass
import concourse.bacc as bacc
import concourse.mybir as mybir
from concourse.tile import TileContext
from concourse.bass_utils import run_bass_kernel_spmd

B, S, D = 4, 2048, 1024
THETA = 10000.0
P = 128
NT = D // P          # 8 partition-tiles over the 1024 dim
NQ = S // 2          # 1024 queries per core
HB = 512             # query half width
NSLOT = S // P       # 16 key slots
F32 = mybir.dt.float32
BF16 = mybir.dt.bfloat16
SCALE = 1.0 / 32.0   # 1/sqrt(D)


def _build_program():
    nc = bacc.Bacc("TRN2", num_swdge_queues=4)
    inp = {}
    def din(name, shape, dt):
        inp[name] = nc.dram_tensor(name, shape, dt, kind="ExternalInput")
    din("xT", [D, S], BF16)          # full x^T (K/V projections)
    din("xTq", [D, NQ], BF16)        # my query columns of x^T
    din("WqT", [D, D], BF16)         # row-permuted (even|odd), transposed
    din("WkT", [D, D], BF16)
    din("WvT", [D, D], BF16)
    din("WoT", [D, D], BF16)
    din("cosK", [D // 2, S], BF16)
    din("sinK", [D // 2, S], BF16)
    din("cosQ", [D // 2, NQ], BF16)
    din("sinQ", [D // 2, NQ], BF16)
    din("mask", [P, 4, 256], BF16)   # diagonal masks, d = slot mod 4
    din("ones_col", [P, 1], BF16)
    din("ones_row", [1, P], BF16)
    outT = nc.dram_tensor("outT", [P, NT, NQ], F32, kind="ExternalOutput")

    xT_r = inp["xT"].rearrange("(t p) s -> p t s", p=P)
    xTq_r = inp["xTq"].rearrange("(t p) s -> p t s", p=P)
    W_r = {w: inp[w].rearrange("(t p) o -> p t o", p=P)
           for w in ("WqT", "WkT", "WvT", "WoT")}
    cosK_r = inp["cosK"].rearrange("(t p) s -> p t s", p=P)
    sinK_r = inp["sinK"].rearrange("(t p) s -> p t s", p=P)
    cosQ_r = inp["cosQ"].rearrange("(t p) s -> p t s", p=P)
    sinQ_r = inp["sinQ"].rearrange("(t p) s -> p t s", p=P)

    from contextlib import ExitStack
    with TileContext(nc) as tc:
        with ExitStack() as ctx:
            pool = lambda *a, **kw: ctx.enter_context(tc.tile_pool(*a, **kw))
            ktp = pool(name="kt", bufs=1)            # KT bf16 resident 32KB
            vp = pool(name="v", bufs=1)              # V bf16 resident 32KB
            qtp = pool(name="qt", bufs=1)            # QT bf16 resident 16KB
            wp = pool(name="w", bufs=2)              # weights 2x16KB
            xbp = pool(name="xb", bufs=4)            # x blocks 4x8KB
            csp = pool(name="cs", bufs=4)            # cos/sin 4x4KB
            ptp = pool(name="pt", bufs=1)            # exp(scores) 16KB
            atp = pool(name="at", bufs=1)            # attnT 8KB
            outp = pool(name="outb", bufs=3)         # out evictions 3x2KB
            smp = pool(name="small", bufs=1)
            tmpp = pool(name="tmp", bufs=2)          # rope tmps 2x2KB
            bcp = pool(name="bc", bufs=3)
            psA = pool(name="psA", bufs=3, space="PSUM")   # proj + scores + bc
            psM = pool(name="psM", bufs=4, space="PSUM")   # PV accum + Wo
            psSum = pool(name="psSum", bufs=1, space="PSUM")

            KT = ktp.tile([P, NT, S], BF16, tag="kt")
            V = vp.tile([P, NSLOT, D], BF16, tag="v")
            QT = qtp.tile([P, NT, NQ], BF16, tag="qt")

            ones_col = smp.tile([P, 1], BF16, tag="onescol")
            ones_row = smp.tile([1, P], BF16, tag="onesrow")
            mask_t = smp.tile([P, 4, 256], BF16, tag="mask")

            def proj_rope(wtile, xtile, cos_t, sin_t, dstT, dcol):
                # dstT[:, i, dcol] (pair i even) / dstT[:, i+4, dcol] (odd)
                for i in range(NT // 2):
                    psE = psA.tile([P, HB], F32, tag="ps", name=f"psE{i}")
                    for dt_ in range(NT):
                        nc.tensor.matmul(psE[:], wtile[:, dt_, i * P:(i + 1) * P],
                                         xtile[:, dt_, :],
                                         start=(dt_ == 0), stop=(dt_ == NT - 1))
                    psO = psA.tile([P, HB], F32, tag="ps", name=f"psO{i}")
                    for dt_ in range(NT):
                        nc.tensor.matmul(psO[:], wtile[:, dt_, (i + 4) * P:(i + 5) * P],
                                         xtile[:, dt_, :],
                                         start=(dt_ == 0), stop=(dt_ == NT - 1))
                    c = cos_t[:, i, :]
                    s = sin_t[:, i, :]
                    t1 = tmpp.tile([P, HB], F32, tag="t", name=f"t1{i}")
                    nc.vector.tensor_mul(out=t1[:], in0=psO[:], in1=s)
                    nc.vector.tensor_mul(out=dstT[:, i, dcol], in0=psE[:], in1=c)
                    nc.vector.tensor_tensor(dstT[:, i, dcol], dstT[:, i, dcol],
                                            t1[:], mybir.AluOpType.subtract)
                    t2 = tmpp.tile([P, HB], F32, tag="t", name=f"t2{i}")
                    nc.vector.tensor_mul(out=t2[:], in0=psE[:], in1=s)
                    nc.vector.tensor_mul(out=dstT[:, i + 4, dcol], in0=psO[:], in1=c)
                    nc.vector.tensor_tensor(dstT[:, i + 4, dcol], dstT[:, i + 4, dcol],
                                            t2[:], mybir.AluOpType.add)

            # ---------- Q projection + RoPE (2 query halves) ----------
            # Spread the critical first loads across engine DMA queues: a
            # sync-queue dma_start issue costs ~0.7us, and serializing 16 of
            # them delays the first matmul by >10us.
            Wq = wp.tile([P, NT, D], BF16, tag="w", name="wq")
            xq0 = xbp.tile([P, NT, HB], BF16, tag="xb", name="xq0")
            # first-consumed chunks lead their own DMA rings: rings process
            # entries in order at ~80GB/s each, so the dt0 pair must not sit
            # behind bulk weight traffic
            nc.gpsimd.dma_start(Wq[:, 0, :], W_r["WqT"][:, 0, :])
            nc.scalar.dma_start(xq0[:, 0, :], xTq_r[:, 0, 0:HB])
            for dt_ in range(1, 4):
                (nc.scalar if dt_ % 2 else nc.gpsimd).dma_start(
                    Wq[:, dt_, :], W_r["WqT"][:, dt_, :])
                (nc.gpsimd if dt_ % 2 else nc.scalar).dma_start(
                    xq0[:, dt_, :], xTq_r[:, dt_, 0:HB])
            for dt_ in range(4, NT):
                nc.sync.dma_start(Wq[:, dt_, :], W_r["WqT"][:, dt_, :])
                nc.sync.dma_start(xq0[:, dt_, :], xTq_r[:, dt_, 0:HB])
            for qh in range(2):
                sl = slice(qh * HB, (qh + 1) * HB)
                if qh == 0:
                    xq = xq0
                else:
                    xq = xbp.tile([P, NT, HB], BF16, tag="xb", name="xq1")
                    nc.sync.dma_start(xq[:], xTq_r[:, :, sl])
                cq = csp.tile([P, NT // 2, HB], BF16, tag="cs", name=f"cq{qh}")
                (nc.scalar if qh == 0 else nc.sync).dma_start(cq[:], cosQ_r[:, :, sl])
                sq = csp.tile([P, NT // 2, HB], BF16, tag="cs", name=f"sq{qh}")
                (nc.gpsimd if qh == 0 else nc.sync).dma_start(sq[:], sinQ_r[:, :, sl])
                proj_rope(Wq, xq, cq, sq, QT, sl)

            # ---------- K projection + RoPE (4 seq blocks) ----------
            Wk = wp.tile([P, NT, D], BF16, tag="w", name="wk")
            nc.sync.dma_start(Wk[:], W_r["WkT"][:, :, :])
            nc.gpsimd.dma_start(ones_col[:], inp["ones_col"][:])
            nc.gpsimd.dma_start(ones_row[:], inp["ones_row"][:])
            nc.gpsimd.dma_start(mask_t[:], inp["mask"][:])
            xbs = []
            for sb in range(4):
                sl = slice(sb * HB, (sb + 1) * HB)
                xb = xbp.tile([P, NT, HB], BF16, tag="xb", name=f"xb{sb}")
                nc.sync.dma_start(xb[:], xT_r[:, :, sl])
                xbs.append(xb)
                ck = csp.tile([P, NT // 2, HB], BF16, tag="cs", name=f"ck{sb}")
                nc.sync.dma_start(ck[:], cosK_r[:, :, sl])
                sk = csp.tile([P, NT // 2, HB], BF16, tag="cs", name=f"sk{sb}")
                nc.sync.dma_start(sk[:], sinK_r[:, :, sl])
                proj_rope(Wk, xb, ck, sk, KT, sl)

            # ---------- V projection (4 seq blocks) ----------
            Wv = wp.tile([P, NT, D], BF16, tag="w", name="wv")
            nc.sync.dma_start(Wv[:], W_r["WvT"][:, :, :])
            for sb in range(4):
                for ss in range(4):
                    for dvb in range(2):
                        ps = psA.tile([P, HB], F32, tag="ps", name=f"psv{sb}{ss}{dvb}")
                        for dt_ in range(NT):
                            nc.tensor.matmul(ps[:], xbs[sb][:, dt_, ss * P:(ss + 1) * P],
                                             Wv[:, dt_, dvb * HB:(dvb + 1) * HB],
                                             start=(dt_ == 0), stop=(dt_ == NT - 1))
                        nc.scalar.copy(V[:, sb * 4 + ss, dvb * HB:(dvb + 1) * HB], ps[:])

            # ---------- Attention + output projection (2 query halves) ----------
            Wo = wp.tile([P, NT, D], BF16, tag="w", name="wo")
            nc.sync.dma_start(Wo[:], W_r["WoT"][:, :, :])

            for ah in range(2):
                nslots = 8 * (ah + 1)
                qsl = slice(ah * HB, (ah + 1) * HB)
                PT = ptp.tile([P, NSLOT, HB], BF16, tag="pt")
                # zero-pad everything left of each masked slot's first
                # possibly-valid column (widest trim is 448 at d=3 partial)
                nc.vector.memset(PT[:, nslots - 8:nslots, 0:448], 0.0)
                # per-key exp sums accumulate on the idle vector engine (a
                # [1,512] sums matmul costs a full 512-row stream on the PE;
                # 24 of them is ~5us) — one cross-partition matmul per half
                # at the end instead
                ptacc = bcp.tile([P, HB], F32, tag="bc", name=f"ptacc{ah}")
                pvA = [psM.tile([P, HB], F32, tag="pv", name=f"pvA{ah}{j}")
                       for j in range(4)]
                for j in range(nslots):
                    masked = j >= nslots - 8
                    partial = j >= nslots - 4
                    base = 256 if partial else 0
                    # leading 64*d columns of a crossing block are fully
                    # above the causal diagonal — skip computing them
                    trim = 64 * (j % 4) if masked else 0
                    qoff = base + trim
                    ps = psA.tile([P, HB], F32, tag="ps", name=f"pss{ah}{j}")
                    pslc = slice(qoff, HB)
                    for dt_ in range(NT):
                        nc.tensor.matmul(ps[:, pslc], KT[:, dt_, j * P:(j + 1) * P],
                                         QT[:, dt_, ah * HB + qoff:(ah + 1) * HB],
                                         start=(dt_ == 0), stop=(dt_ == NT - 1))
                    nc.scalar.activation(PT[:, j, pslc], ps[:, pslc],
                                         mybir.ActivationFunctionType.Exp, scale=SCALE)
                    if masked:
                        nc.vector.tensor_mul(out=PT[:, j, base + trim:base + 256],
                                             in0=PT[:, j, base + trim:base + 256],
                                             in1=mask_t[:, j % 4, trim:256])
                    # trimmed slots (except the last, which closes the psum
                    # group full-width over the zero-padded PT) skip the
                    # above-diagonal columns
                    nsl = pslc if qoff and j < nslots - 1 else slice(0, HB)
                    if j == 0:
                        nc.vector.tensor_copy(ptacc[:], PT[:, 0, :])
                    else:
                        nc.vector.tensor_tensor(ptacc[:, nsl], ptacc[:, nsl],
                                                PT[:, j, nsl], mybir.AluOpType.add)
                    # PV pass A (dv chunks 0-3) rides inside the scores stream
                    for dci in range(4):
                        nc.tensor.matmul(pvA[dci][:, nsl], V[:, j, dci * P:(dci + 1) * P],
                                         PT[:, j, nsl],
                                         start=(j == 0), stop=(j == nslots - 1))

                ptacc_bf = smp.tile([P, HB], BF16, tag="ptaccbf")
                nc.vector.tensor_copy(ptacc_bf[:], ptacc[:])

                attnT = atp.tile([P, NT, HB], BF16, tag="at")
                for dci in range(4):
                    # alternate engines: PV-B matmuls wait on these copies to
                    # free the psM pool, so halve the serial eviction latency
                    if dci % 2:
                        nc.scalar.copy(attnT[:, dci, :], pvA[dci][:])
                    else:
                        nc.vector.tensor_copy(attnT[:, dci, :], pvA[dci][:])
                pvB = [psM.tile([P, HB], F32, tag="pv", name=f"pvB{ah}{j}")
                       for j in range(4)]
                for j in range(nslots):
                    if j >= nslots - 8 and j < nslots - 1:
                        qoffB = (256 if j >= nslots - 4 else 0) + 64 * (j % 4)
                    else:
                        qoffB = 0
                    nsl = slice(qoffB, HB)
                    for dci in range(4):
                        nc.tensor.matmul(pvB[dci][:, nsl], V[:, j, (4 + dci) * P:(5 + dci) * P],
                                         PT[:, j, nsl],
                                         start=(j == 0), stop=(j == nslots - 1))
                for dci in range(4):
                    # alternate engines (as for PV-A): Wo oc0's last dc
                    # accumulation waits on the final attnT copy
                    if dci % 2:
                        nc.scalar.copy(attnT[:, 4 + dci, :], pvB[dci][:])
                    else:
                        nc.vector.tensor_copy(attnT[:, 4 + dci, :], pvB[dci][:])

                # cross-partition reduce of the per-key sums (placed after
                # PV-B: the PE must not wait on the last vector add); the
                # broadcast-then-reciprocal chain hides under Wo's first oc
                # matmuls ([1,512] reciprocal runs serial on one DVE lane)
                sums = psSum.tile([1, HB], F32, tag="sums")
                nc.tensor.matmul(sums[:], ones_col[:], ptacc_bf[:], start=True, stop=True)
                sums_sb = smp.tile([1, HB], BF16, tag="sums_sb")
                nc.scalar.copy(sums_sb[:], sums[:])

                bc = None
                wops = []
                for oc in range(NT):
                    ps = psM.tile([P, HB], F32, tag="pv", name=f"wo{ah}{oc}")
                    for dc in range(NT):
                        nc.tensor.matmul(ps[:], Wo[:, dc, oc * P:(oc + 1) * P],
                                         attnT[:, dc, :],
                                         start=(dc == 0), stop=(dc == NT - 1))
                    wops.append(ps)
                    if oc == 0:
                        bc_ps = psA.tile([P, HB], F32, tag="ps", name=f"bc{ah}")
                        nc.tensor.matmul(bc_ps[:], ones_row[:], sums_sb[:],
                                         start=True, stop=True)
                        bcs = bcp.tile([P, HB], F32, tag="bc", name=f"bcs{ah}")
                        nc.scalar.copy(bcs[:], bc_ps[:])
                        bc = bcp.tile([P, HB], F32, tag="bc", name=f"bcr{ah}")
                        nc.vector.reciprocal(bc[:], bcs[:])
                    else:
                        pso = wops[oc - 1]
                        ot = outp.tile([P, HB], F32, tag="outb")
                        nc.vector.tensor_mul(out=ot[:], in0=pso[:], in1=bc[:])
                        nc.sync.dma_start(outT[:, oc - 1, qsl], ot[:])
                ot = outp.tile([P, HB], F32, tag="outb")
                nc.vector.tensor_mul(out=ot[:], in0=wops[NT - 1][:], in1=bc[:])
                nc.sync.dma_start(outT[:, NT - 1, qsl], ot[:])

    nc.finalize()
    return nc


def _host_inputs(x, Wq, Wk, Wv, Wo, token_positions):
    import ml_dtypes
    bf = ml_dtypes.bfloat16
    perm = np.concatenate([np.arange(0, D, 2), np.arange(1, D, 2)])
    WqTp = np.ascontiguousarray(Wq[perm].T.astype(bf))
    WkTp = np.ascontiguousarray(Wk[perm].T.astype(bf))
    WvT = np.ascontiguousarray(Wv.T.astype(bf))
    WoT = np.ascontiguousarray(Wo.T.astype(bf))
    inv_freq = (1.0 / (np.float32(THETA) **
                       (np.arange(0, D, 2, dtype=np.float32) / np.float32(D))))

    in_maps, metas = [], []
    for b in range(B):
        xT = np.ascontiguousarray(x[b].T.astype(bf))   # [D, S]
        pos = token_positions[b].astype(np.float32)
        ang = (pos[None, :] * inv_freq[:, None]).astype(np.float32)  # [D/2, S]
        cosF = np.cos(ang)
        sinF = np.sin(ang)
        for h in range(2):
            qcols = np.arange(h, S, 2)
            xTq = np.ascontiguousarray(xT[:, qcols])
            cosQ = np.ascontiguousarray(cosF[:, qcols].astype(bf))
            sinQ = np.ascontiguousarray(sinF[:, qcols].astype(bf))
            # mask[p, d, i] = (2i + h >= 128d + p), query-tile independent
            i_ = np.arange(256)
            p_ = np.arange(P)
            d_ = np.arange(4)
            m = ((2 * i_[None, None, :] + h) >=
                 (128 * d_[None, :, None] + p_[:, None, None]))
            in_maps.append({
                "xT": xT, "xTq": xTq,
                "WqT": WqTp, "WkT": WkTp, "WvT": WvT, "WoT": WoT,
                "cosK": cosF.astype(bf), "sinK": sinF.astype(bf),
                "cosQ": cosQ, "sinQ": sinQ,
                "mask": m.astype(bf),
                "ones_col": np.ones((P, 1), bf),
                "ones_row": np.ones((1, P), bf),
            })
            metas.append((b, qcols))
    return in_maps, metas


_NC_CACHE = {}


def kernel(x, Wq, Wk, Wv, Wo, token_positions):
    x = np.asarray(x); token_positions = np.asarray(token_positions)
    if "nc" not in _NC_CACHE:
        _NC_CACHE["nc"] = _build_program()
    nc = _NC_CACHE["nc"]
    in_maps, metas = _host_inputs(np.asarray(x), np.asarray(Wq), np.asarray(Wk),
                                  np.asarray(Wv), np.asarray(Wo), token_positions)
    res = run_bass_kernel_spmd(nc, in_maps, core_ids=list(range(8)))
    out = np.empty((B, S, D), dtype=np.float32)
    for (b, qcols), r in zip(metas, res.results):
        oT = r["outT"]                       # [P, NT, NQ]
        o = np.transpose(oT, (2, 1, 0)).reshape(NQ, D)
        out[b, qcols, :] = o
    return out
